# revision 9
# baseline (speedup 1.0000x reference)
"""MoE MLP (top-2 routing, 8 experts) on 8 Trainium2 NeuronCores.

Strategy: expert-parallel along the *hidden* (F) axis instead of the
expert axis. The old one-expert-per-core split is load-imbalanced: the
busiest expert gets 2175 of the 16384 token-expert pairs while the
average is 2048, and exec time is the max over cores, so every core
pays the straggler's 6%. Here every core owns a 512-wide F-slice of
ALL 8 experts (W_in[:, :, sl], W_out[:, sl, :]) and processes ALL
routed tokens, producing a partial y that the host sums across cores.
Per-core work is identical by construction: 16384 tokens x (1024x512)
x 2 matmuls = 437us of fp16 PE roofline (vs 465us before).

The router (0.05% of FLOPs) runs on the host, which doubles as the
dispatch: tokens are gathered per selected expert into one [D, 16384]
fp16 stream shared by all cores. Each core runs the fused MLP

    y_partial = W_out[sl,:]^T @ gelu(W_in[:,sl]^T @ x + b_in[sl])

in a transposed layout (tokens on the free axis) so weights stay
stationary on the PE array. Combine weights and b_out are applied on
the host during the partial-sum reduction (free), so the device does
no per-token scaling at all. Weight slices stream per-expert (2MB
each) on the sync DMA ring two experts ahead; x chunks ride the
scalar ring; y partials (fp16) interleave on the sync ring.

fp8 was evaluated and rejected: e4m3 quantization measures 4.6e-2
rel-max error on this problem (gate is 2e-2); every partial-fp8
scheme also fails. fp16 measures ~4e-4.
"""

import contextlib
import ctypes
import os
import sys
import types
from contextlib import ExitStack

import numpy as np

import concourse.bass as bass
import concourse.mybir as mybir
import concourse.tile as tile
from concourse import bacc
from concourse.bass_utils import run_bass_kernel_spmd


def _install_ntff_hook():
    """Provide antenv.axon_hooks (absent in this image) so BASS_TRACE=1
    can capture NTFF profiles through the axon PJRT .so. No-op if the
    module already exists or the .so/symbols are unavailable."""
    try:
        from antenv.axon_hooks import get_axon_ntff_profile_hook  # noqa: F401
        return
    except ImportError:
        pass
    so_path = "/opt/axon/libaxon_pjrt.so"
    if not os.path.exists(so_path):
        return
    try:
        lib = ctypes.CDLL(so_path)
    except OSError:
        return
    if not hasattr(lib, "axon_start_nrt_profile"):
        return
    lib.axon_start_nrt_profile.argtypes = [
        ctypes.POINTER(ctypes.c_int64), ctypes.c_size_t]
    lib.axon_start_nrt_profile.restype = ctypes.c_int64
    lib.axon_stop_nrt_profile.argtypes = [ctypes.c_char_p]
    lib.axon_stop_nrt_profile.restype = ctypes.c_int64

    @contextlib.contextmanager
    def _hook(output_dir, device_ids):
        import jax
        jax.devices()  # force PJRT init so the .so's client exists
        if device_ids:
            ids = (ctypes.c_int64 * len(device_ids))(*device_ids)
            rc = lib.axon_start_nrt_profile(ids, len(device_ids))
        else:
            rc = lib.axon_start_nrt_profile(None, 0)
        if rc != 0:
            raise RuntimeError(f"axon_start_nrt_profile rc={rc}")
        try:
            yield
        finally:
            n = lib.axon_stop_nrt_profile(str(output_dir).encode())
            print(f"ntff profile: {n} file(s) -> {output_dir}", file=sys.stderr)

    import antenv
    mod = types.ModuleType("antenv.axon_hooks")
    mod.get_axon_ntff_profile_hook = lambda: _hook
    mod.set_axon_ntff_profile_hook = lambda h: None
    sys.modules["antenv.axon_hooks"] = mod
    antenv.axon_hooks = mod

B, S, D, F, E = 4, 2048, 1024, 4096, 8
T = B * S
TOP_K = 2
NCORES = 8
P = 128
FS = F // NCORES          # 512-wide F-slice per core
ND, NB = D // P, FS // P  # 8 d-tiles, 4 f-tiles per slice

# test.py pokes these for profiling info
LAST_RESULT = None

_cache = {}


def _chunk_list(C):
    """Token chunks (PSUM free-dim <= 512). Chunks below 256 run
    LDWEIGHTS-bound on the PE, so a short tail is split off the
    previous 512 chunk into two >=256 pieces instead."""
    chunks = [512] * (C // 512)
    rem = C % 512
    if rem:
        if rem < 256 and chunks:
            tot = 512 + rem
            a = tot // 2
            chunks[-1] = a
            chunks.append(tot - a)
        else:
            chunks.append(rem)
    return chunks


def _chunk_plan(counts):
    """[(expert, global_off, size, first_of_expert)], with a small ramp
    chunk up front (fast time-to-first-matmul) and a small tail chunk
    (short drain after the last matmul)."""
    lists = [_chunk_list(c) for c in counts]
    for lst in lists:
        if lst:
            if lst[0] >= 512:  # startup ramp: 128+384 instead of 512
                lst[0] = 384
                lst.insert(0, 128)
            break
    for lst in reversed(lists):
        if lst:
            if lst[-1] > 192:  # tail: split off a final 128
                lst[-1] -= 128
                lst.append(128)
            break
    plan = []
    off = 0
    for e, lst in enumerate(lists):
        for i, ck in enumerate(lst):
            plan.append((e, off, ck, i == 0))
            off += ck
    return plan, off


def _build_bass(counts):
    dt = mybir.dt
    io_dt = dt.float16
    plan, CT = _chunk_plan(counts)
    nc = bacc.Bacc("TRN2", target_bir_lowering=False, debug=False)

    x8 = nc.dram_tensor("x8", [D, CT], io_dt, kind="ExternalInput")
    win8 = nc.dram_tensor("win8", [D, E, FS], io_dt, kind="ExternalInput")
    wout8 = nc.dram_tensor("wout8", [FS, E, D], io_dt, kind="ExternalInput")
    # host pre-arranges b_in slice as [p, e*fc] so the DMA is contiguous
    bin8 = nc.dram_tensor("bin8", [P, E * NB], dt.float32, kind="ExternalInput")
    y8 = nc.dram_tensor("y8", [D, CT], io_dt, kind="ExternalOutput")

    x_r = x8.ap().rearrange("(dn p) c -> p dn c", p=P)
    win_r = win8.ap().rearrange("(dn p) e f -> p dn e f", p=P)
    wout_r = wout8.ap().rearrange("(fb p) e d -> p fb e d", p=P)
    y_r = y8.ap().rearrange("(dn p) c -> p dn c", p=P)

    with tile.TileContext(nc) as tc, ExitStack() as ctx:
        consts = ctx.enter_context(tc.tile_pool(name="consts", bufs=1))
        xpool = ctx.enter_context(tc.tile_pool(name="x", bufs=3))
        winpool = ctx.enter_context(tc.tile_pool(name="win", bufs=3))
        woutpool = ctx.enter_context(tc.tile_pool(name="wout", bufs=3))
        hpool = ctx.enter_context(tc.tile_pool(name="h", bufs=2))
        ypool = ctx.enter_context(tc.tile_pool(name="y", bufs=16))
        psum_h = ctx.enter_context(tc.tile_pool(name="ph", bufs=3, space="PSUM"))
        psum_y = ctx.enter_context(tc.tile_pool(name="py", bufs=3, space="PSUM"))

        def x_dma(e_off_ck):
            _, off, ck, _ = e_off_ck
            x_t = xpool.tile([P, ND, ck], io_dt, tag="x")
            nc.scalar.dma_start(x_t[:], x_r[:, :, off:off + ck])
            return x_t

        def w_dma(e):
            win_t = winpool.tile([P, ND, FS], io_dt, tag="win")
            nc.sync.dma_start(win_t[:], win_r[:, :, e, :])
            wout_t = woutpool.tile([P, NB, D], io_dt, tag="wout")
            nc.sync.dma_start(wout_t[:], wout_r[:, :, e, :])
            return win_t, wout_t

        # Critical path: x chunk 0 (scalar ring) and expert 0's first
        # W_in stripe (sync ring; fc0 split out so the very first
        # matmul group only waits on 0.25 MB) go first on their rings.
        x0_t = xpool.tile([P, ND, plan[0][2]], io_dt, tag="x", name="x0")
        nc.scalar.dma_start(x0_t[:], x_r[:, :, 0:plan[0][2]])
        x_pref = [x0_t]
        win0_t = winpool.tile([P, ND, FS], io_dt, tag="win")
        nc.sync.dma_start(win0_t[:, :, :P], win_r[:, :, 0, :P])
        nc.sync.dma_start(win0_t[:, :, P:], win_r[:, :, 0, P:])
        wout0_t = woutpool.tile([P, NB, D], io_dt, tag="wout")
        nc.sync.dma_start(wout0_t[:], wout_r[:, :, 0, :])
        w_tiles = {0: (win0_t, wout0_t)}

        # b_in is tiny and needed by the first gelu — SWDGE queue.
        bin_t = consts.tile([P, E * NB], dt.float32)
        nc.gpsimd.dma_start(bin_t[:], bin8.ap())

        # Prefetch: x one chunk ahead, weights one expert ahead.
        if len(plan) > 1:
            x_pref.append(x_dma(plan[1]))
        if E > 1 and counts[1]:
            w_tiles[1] = w_dma(1)

        def m1(ci):
            # phase A: h = gelu(W_in^T @ x + b_in), laid out [f, tok]
            e, off, ck, first = plan[ci]
            x_t = x_pref.pop(0)
            if ci + 2 < len(plan):
                x_pref.append(x_dma(plan[ci + 2]))
            if e not in w_tiles:  # fallback for degenerate chunk plans
                w_tiles[e] = w_dma(e)
            win_t = w_tiles[e][0]
            h_t = hpool.tile([P, NB, ck], io_dt, tag="h", name="h_t")
            for fc in range(NB):
                ph = psum_h.tile([P, ck], dt.float32, tag="ph", name="ph")
                for dn in range(ND):
                    nc.tensor.matmul(
                        ph[:],
                        win_t[:, dn, fc * P:(fc + 1) * P],
                        x_t[:, dn, :],
                        start=(dn == 0),
                        stop=(dn == ND - 1),
                    )
                nc.scalar.activation(
                    h_t[:, fc, :], ph[:],
                    mybir.ActivationFunctionType.Gelu,
                    bias=bin_t[:, e * NB + fc:e * NB + fc + 1],
                )
            return h_t

        def m2(ci, h_t):
            # phase B: y_partial = W_out^T @ h, laid out [d, tok]
            e, off, ck, first = plan[ci]
            if not first and plan[ci - 1][3] and e + 2 < E and counts[e + 2] \
                    and (e + 2) not in w_tiles:
                # second chunk of expert e: prefetch expert e+2's weights
                # (queued behind the first chunk's y DMAs on the sync ring)
                w_tiles[e + 2] = w_dma(e + 2)
            wout_t = w_tiles[e][1]
            last = ci >= len(plan) - 2  # drain tail on both rings
            for dn in range(ND):
                py = psum_y.tile([P, ck], dt.float32, tag="py", name="py")
                for fb in range(NB):
                    nc.tensor.matmul(
                        py[:],
                        wout_t[:, fb, dn * P:(dn + 1) * P],
                        h_t[:, fb, :],
                        start=(fb == 0),
                        stop=(fb == NB - 1),
                    )
                y_t = ypool.tile([P, ck], io_dt, tag="y", name="y_t")
                nc.vector.tensor_copy(y_t[:], py[:])
                eng = nc.scalar if (last and dn % 2) else nc.sync
                eng.dma_start(y_r[:, dn, off:off + ck], y_t[:])

        # Software-pipeline m1 one chunk ahead of m2: m2(k) overlaps
        # m1(k+1)'s matmuls and covers each expert's W_out DMA latency
        # (chunk 0's m2 would otherwise stall on wout0 at startup).
        h_prev = m1(0)
        for ci in range(len(plan)):
            h_cur = h_prev
            if ci + 1 < len(plan):
                h_prev = m1(ci + 1)
            m2(ci, h_cur)

    nc.compile()
    return nc, CT


def _get_nc(counts):
    key = tuple(counts)
    if key not in _cache:
        _cache[key] = _build_bass(counts)
    return _cache[key]


def _route(x, W_router):
    """Host-side router: top-2 selection + renormalized weights (fp64).

    Matches jax.lax.top_k on softmax(logits): softmax is monotone so
    top-2 of logits is identical, with ties broken toward lower index
    (argsort stable on -logits).
    """
    lg = x.astype(np.float64) @ W_router.T.astype(np.float64)
    top2 = np.argsort(-lg, axis=1, kind="stable")[:, :TOP_K]
    l1 = np.take_along_axis(lg, top2[:, 0:1], 1)
    l2 = np.take_along_axis(lg, top2[:, 1:2], 1)
    e2 = np.exp(l2 - l1)
    w1 = (1.0 / (1.0 + e2)).astype(np.float32)
    w2 = (e2 / (1.0 + e2)).astype(np.float32)
    return top2, np.concatenate([w1, w2], axis=1)


def kernel(residual, W_router, W_in, b_in, W_out, b_out):
    global LAST_RESULT

    x = np.ascontiguousarray(np.asarray(residual, dtype=np.float32).reshape(T, D))
    W_in = np.asarray(W_in, dtype=np.float32)
    W_out = np.asarray(W_out, dtype=np.float32)
    b_in = np.asarray(b_in, dtype=np.float32)
    b_out = np.asarray(b_out, dtype=np.float32)

    top2, wts = _route(x, np.asarray(W_router, dtype=np.float32))

    idxs, ws = [], []
    for e in range(E):
        sel0 = top2[:, 0] == e
        sel1 = top2[:, 1] == e
        idx = np.concatenate([np.where(sel0)[0], np.where(sel1)[0]])
        w = np.concatenate([wts[sel0, 0], wts[sel1, 1]])
        idxs.append(idx)
        ws.append(w)
    counts = [len(i) for i in idxs]

    nc, CT = _get_nc(counts)

    # One shared token stream: all experts' gathered tokens, transposed
    # to [D, CT] fp16 (the per-expert order matches _chunk_plan's).
    order = np.concatenate(idxs)
    x8 = np.ascontiguousarray(x[order].T.astype(np.float16))
    assert x8.shape[1] == CT

    in_maps = []
    for c in range(NCORES):
        sl = slice(c * FS, (c + 1) * FS)
        in_maps.append({
            "x8": x8,
            "win8": np.ascontiguousarray(
                W_in[:, :, sl].transpose(1, 0, 2).astype(np.float16)),
            "wout8": np.ascontiguousarray(
                W_out[:, sl, :].transpose(1, 0, 2).astype(np.float16)),
            "bin8": np.ascontiguousarray(
                b_in[:, sl].reshape(E, NB, P).transpose(2, 0, 1).reshape(P, E * NB)),
        })

    if os.environ.get("BASS_TRACE"):
        _install_ntff_hook()
    LAST_RESULT = run_bass_kernel_spmd(nc, in_maps, list(range(NCORES)))

    # Host reduction: sum the 8 F-slice partials, add b_out, apply the
    # renormalized top-2 combine weights, scatter-add into [T, D].
    Y = np.zeros((D, CT), dtype=np.float32)
    for c in range(NCORES):
        Y += LAST_RESULT.results[c]["y8"].astype(np.float32)

    y = np.zeros((T, D), dtype=np.float32)
    off = 0
    for e in range(E):
        cnt = counts[e]
        cols = Y[:, off:off + cnt] + b_out[e][:, None]
        y[idxs[e]] += (cols * ws[e][None, :]).T
        off += cnt
    return y.reshape(B, S, D)


# revision 16
# speedup vs baseline: 1.0116x; 1.0116x over previous
"""MoE MLP (top-2 routing, 8 experts) on 8 Trainium2 NeuronCores.

Strategy: 2 token groups x 4-way F-split (hybrid of data- and
tensor-parallel over the expert MLPs). One-expert-per-core is
load-imbalanced (busiest expert 2175 vs mean 2048 of the 16384
token-expert pairs; exec time is the max over cores). Pure 8-way
F-split balances perfectly but ships the full 16384-token stream and
a full-size y partial to/from every core — the ~900MB/call of
host<->device traffic overlaps neighboring cores' execution on shared
HBM and produced 10-50us random stragglers. The G=2 x S=4 hybrid
halves both: cores 0-3 process token group A against F-slices
0-3 (1024 wide) of group A's 4 experts; cores 4-7 likewise for
group B. Experts are assigned to groups by sorted load (alternating),
and slot capacities are the pairwise max, so both groups share ONE
chunk plan (SPMD requires a single program) with only ~1% padding:
8285 tokens/core x (1024x1024) x 2 matmuls ~= 442us fp16 PE roofline.

The router (0.05% of FLOPs) runs on the host, which doubles as the
dispatch: tokens are gathered per selected expert into a [D, 8285]
fp16 stream per group. Each core runs the fused MLP

    y_partial = W_out[sl,:]^T @ gelu(W_in[:,sl]^T @ x + b_in[sl])

in a transposed layout (tokens on the free axis) so weights stay
stationary on the PE array. Combine weights and b_out are applied on
the host during the partial-sum reduction (free), so the device does
no per-token scaling at all. Weight slot slices (4MB) stream on the
sync DMA ring two slots ahead; x chunks ride the scalar ring; y
partials (fp16) interleave on the sync ring.

fp8 was evaluated and rejected: e4m3 quantization measures 4.6e-2
rel-max error on this problem (gate is 2e-2); every partial-fp8
scheme also fails. fp16 measures ~4e-4.
"""

import contextlib
import ctypes
import os
import sys
import types
from contextlib import ExitStack

import numpy as np

import concourse.bass as bass
import concourse.mybir as mybir
import concourse.tile as tile
from concourse import bacc
from concourse.bass_utils import run_bass_kernel_spmd


def _install_ntff_hook():
    """Provide antenv.axon_hooks (absent in this image) so BASS_TRACE=1
    can capture NTFF profiles through the axon PJRT .so. No-op if the
    module already exists or the .so/symbols are unavailable."""
    try:
        from antenv.axon_hooks import get_axon_ntff_profile_hook  # noqa: F401
        return
    except ImportError:
        pass
    so_path = "/opt/axon/libaxon_pjrt.so"
    if not os.path.exists(so_path):
        return
    try:
        lib = ctypes.CDLL(so_path)
    except OSError:
        return
    if not hasattr(lib, "axon_start_nrt_profile"):
        return
    lib.axon_start_nrt_profile.argtypes = [
        ctypes.POINTER(ctypes.c_int64), ctypes.c_size_t]
    lib.axon_start_nrt_profile.restype = ctypes.c_int64
    lib.axon_stop_nrt_profile.argtypes = [ctypes.c_char_p]
    lib.axon_stop_nrt_profile.restype = ctypes.c_int64

    @contextlib.contextmanager
    def _hook(output_dir, device_ids):
        import jax
        jax.devices()  # force PJRT init so the .so's client exists
        if device_ids:
            ids = (ctypes.c_int64 * len(device_ids))(*device_ids)
            rc = lib.axon_start_nrt_profile(ids, len(device_ids))
        else:
            rc = lib.axon_start_nrt_profile(None, 0)
        if rc != 0:
            raise RuntimeError(f"axon_start_nrt_profile rc={rc}")
        try:
            yield
        finally:
            n = lib.axon_stop_nrt_profile(str(output_dir).encode())
            print(f"ntff profile: {n} file(s) -> {output_dir}", file=sys.stderr)

    import antenv
    mod = types.ModuleType("antenv.axon_hooks")
    mod.get_axon_ntff_profile_hook = lambda: _hook
    mod.set_axon_ntff_profile_hook = lambda h: None
    sys.modules["antenv.axon_hooks"] = mod
    antenv.axon_hooks = mod

B, S, D, F, E = 4, 2048, 1024, 4096, 8
T = B * S
TOP_K = 2
NCORES = 8
P = 128
G, NS = 2, 4              # 2 token groups x 4-way F-split (4 expert slots/core)
FS = F // NS              # 1024-wide F-slice per core
ND, NB = D // P, FS // P  # 8 d-tiles, 8 f-tiles per slice

# test.py pokes these for profiling info
LAST_RESULT = None

_cache = {}


def _chunk_list(C):
    """Token chunks (PSUM free-dim <= 512). Chunks below 256 run
    LDWEIGHTS-bound on the PE, so a short tail is split off the
    previous 512 chunk into two >=256 pieces instead."""
    chunks = [512] * (C // 512)
    rem = C % 512
    if rem:
        if rem < 256 and chunks:
            tot = 512 + rem
            a = tot // 2
            chunks[-1] = a
            chunks.append(tot - a)
        else:
            chunks.append(rem)
    return chunks


def _chunk_plan(counts):
    """[(expert, global_off, size, first_of_expert)], with a small ramp
    chunk up front (fast time-to-first-matmul) and a small tail chunk
    (short drain after the last matmul)."""
    lists = [_chunk_list(c) for c in counts]
    for lst in lists:
        if lst:
            if lst[0] >= 512:  # startup ramp: 128+384 instead of 512
                lst[0] = 384
                lst.insert(0, 128)
            break
    for lst in reversed(lists):
        if lst:
            if lst[-1] > 192:  # tail: split off a final 128
                lst[-1] -= 128
                lst.append(128)
            break
    plan = []
    off = 0
    for e, lst in enumerate(lists):
        for i, ck in enumerate(lst):
            plan.append((e, off, ck, i == 0))
            off += ck
    return plan, off


def _build_bass(counts):
    dt = mybir.dt
    io_dt = dt.float16
    plan, CT = _chunk_plan(counts)
    nc = bacc.Bacc("TRN2", target_bir_lowering=False, debug=False)

    x8 = nc.dram_tensor("x8", [D, CT], io_dt, kind="ExternalInput")
    win8 = nc.dram_tensor("win8", [D, NS, FS], io_dt, kind="ExternalInput")
    wout8 = nc.dram_tensor("wout8", [FS, NS, D], io_dt, kind="ExternalInput")
    # host pre-arranges b_in slice as [p, slot*fc] so the DMA is contiguous
    bin8 = nc.dram_tensor("bin8", [P, NS * NB], dt.float32, kind="ExternalInput")
    y8 = nc.dram_tensor("y8", [D, CT], io_dt, kind="ExternalOutput")

    x_r = x8.ap().rearrange("(dn p) c -> p dn c", p=P)
    win_r = win8.ap().rearrange("(dn p) e f -> p dn e f", p=P)
    wout_r = wout8.ap().rearrange("(fb p) e d -> p fb e d", p=P)
    y_r = y8.ap().rearrange("(dn p) c -> p dn c", p=P)

    with tile.TileContext(nc) as tc, ExitStack() as ctx:
        consts = ctx.enter_context(tc.tile_pool(name="consts", bufs=1))
        xpool = ctx.enter_context(tc.tile_pool(name="x", bufs=3))
        winpool = ctx.enter_context(tc.tile_pool(name="win", bufs=3))
        woutpool = ctx.enter_context(tc.tile_pool(name="wout", bufs=3))
        hpool = ctx.enter_context(tc.tile_pool(name="h", bufs=2))
        ypool = ctx.enter_context(tc.tile_pool(name="y", bufs=16))
        psum_h = ctx.enter_context(tc.tile_pool(name="ph", bufs=3, space="PSUM"))
        psum_y = ctx.enter_context(tc.tile_pool(name="py", bufs=3, space="PSUM"))

        def x_dma(e_off_ck):
            _, off, ck, _ = e_off_ck
            x_t = xpool.tile([P, ND, ck], io_dt, tag="x")
            nc.scalar.dma_start(x_t[:], x_r[:, :, off:off + ck])
            return x_t

        def w_dma(e):
            win_t = winpool.tile([P, ND, FS], io_dt, tag="win")
            nc.sync.dma_start(win_t[:], win_r[:, :, e, :])
            wout_t = woutpool.tile([P, NB, D], io_dt, tag="wout")
            nc.sync.dma_start(wout_t[:], wout_r[:, :, e, :])
            return win_t, wout_t

        # Critical path: x chunk 0 (scalar ring) and expert 0's first
        # W_in stripe (sync ring; fc0 split out so the very first
        # matmul group only waits on 0.25 MB) go first on their rings.
        x0_t = xpool.tile([P, ND, plan[0][2]], io_dt, tag="x", name="x0")
        nc.scalar.dma_start(x0_t[:], x_r[:, :, 0:plan[0][2]])
        x_pref = [x0_t]
        win0_t = winpool.tile([P, ND, FS], io_dt, tag="win")
        nc.sync.dma_start(win0_t[:, :, :P], win_r[:, :, 0, :P])
        nc.sync.dma_start(win0_t[:, :, P:], win_r[:, :, 0, P:])
        wout0_t = woutpool.tile([P, NB, D], io_dt, tag="wout")
        nc.sync.dma_start(wout0_t[:], wout_r[:, :, 0, :])
        w_tiles = {0: (win0_t, wout0_t)}

        # b_in is tiny and needed by the first gelu — SWDGE queue.
        bin_t = consts.tile([P, NS * NB], dt.float32)
        nc.gpsimd.dma_start(bin_t[:], bin8.ap())

        # Prefetch: x one chunk ahead, weights one slot ahead.
        if len(plan) > 1:
            x_pref.append(x_dma(plan[1]))
        if NS > 1 and counts[1]:
            w_tiles[1] = w_dma(1)

        def m1(ci):
            # phase A: h = gelu(W_in^T @ x + b_in), laid out [f, tok]
            e, off, ck, first = plan[ci]
            x_t = x_pref.pop(0)
            if ci + 2 < len(plan):
                x_pref.append(x_dma(plan[ci + 2]))
            if e not in w_tiles:  # fallback for degenerate chunk plans
                w_tiles[e] = w_dma(e)
            win_t = w_tiles[e][0]
            h_t = hpool.tile([P, NB, ck], io_dt, tag="h", name="h_t")
            for fc in range(NB):
                ph = psum_h.tile([P, ck], dt.float32, tag="ph", name="ph")
                for dn in range(ND):
                    nc.tensor.matmul(
                        ph[:],
                        win_t[:, dn, fc * P:(fc + 1) * P],
                        x_t[:, dn, :],
                        start=(dn == 0),
                        stop=(dn == ND - 1),
                    )
                nc.scalar.activation(
                    h_t[:, fc, :], ph[:],
                    mybir.ActivationFunctionType.Gelu,
                    bias=bin_t[:, e * NB + fc:e * NB + fc + 1],
                )
            return h_t

        def m2(ci, h_t):
            # phase B: y_partial = W_out^T @ h, laid out [d, tok]
            e, off, ck, first = plan[ci]
            if not first and plan[ci - 1][3] and e + 2 < NS and counts[e + 2] \
                    and (e + 2) not in w_tiles:
                # second chunk of slot e: prefetch slot e+2's weights
                # (queued behind the first chunk's y DMAs on the sync ring)
                w_tiles[e + 2] = w_dma(e + 2)
            wout_t = w_tiles[e][1]
            last = ci >= len(plan) - 2  # drain tail on both rings
            for dn in range(ND):
                py = psum_y.tile([P, ck], dt.float32, tag="py", name="py")
                for fb in range(NB):
                    nc.tensor.matmul(
                        py[:],
                        wout_t[:, fb, dn * P:(dn + 1) * P],
                        h_t[:, fb, :],
                        start=(fb == 0),
                        stop=(fb == NB - 1),
                    )
                y_t = ypool.tile([P, ck], io_dt, tag="y", name="y_t")
                nc.vector.tensor_copy(y_t[:], py[:])
                eng = nc.scalar if (last and dn % 2) else nc.sync
                eng.dma_start(y_r[:, dn, off:off + ck], y_t[:])

        # Software-pipeline m1 one chunk ahead of m2: m2(k) overlaps
        # m1(k+1)'s matmuls and covers each expert's W_out DMA latency
        # (chunk 0's m2 would otherwise stall on wout0 at startup).
        h_prev = m1(0)
        for ci in range(len(plan)):
            h_cur = h_prev
            if ci + 1 < len(plan):
                h_prev = m1(ci + 1)
            m2(ci, h_cur)

    nc.compile()
    return nc, CT


def _get_nc(counts):
    key = tuple(counts)
    if key not in _cache:
        _cache[key] = _build_bass(counts)
    return _cache[key]


def _route(x, W_router):
    """Host-side router: top-2 selection + renormalized weights (fp64).

    Matches jax.lax.top_k on softmax(logits): softmax is monotone so
    top-2 of logits is identical, with ties broken toward lower index
    (argsort stable on -logits).
    """
    lg = x.astype(np.float64) @ W_router.T.astype(np.float64)
    top2 = np.argsort(-lg, axis=1, kind="stable")[:, :TOP_K]
    l1 = np.take_along_axis(lg, top2[:, 0:1], 1)
    l2 = np.take_along_axis(lg, top2[:, 1:2], 1)
    e2 = np.exp(l2 - l1)
    w1 = (1.0 / (1.0 + e2)).astype(np.float32)
    w2 = (e2 / (1.0 + e2)).astype(np.float32)
    return top2, np.concatenate([w1, w2], axis=1)


def kernel(residual, W_router, W_in, b_in, W_out, b_out):
    global LAST_RESULT

    x = np.ascontiguousarray(np.asarray(residual, dtype=np.float32).reshape(T, D))
    W_in = np.asarray(W_in, dtype=np.float32)
    W_out = np.asarray(W_out, dtype=np.float32)
    b_in = np.asarray(b_in, dtype=np.float32)
    b_out = np.asarray(b_out, dtype=np.float32)

    top2, wts = _route(x, np.asarray(W_router, dtype=np.float32))

    idxs, ws = [], []
    for e in range(E):
        sel0 = top2[:, 0] == e
        sel1 = top2[:, 1] == e
        idx = np.concatenate([np.where(sel0)[0], np.where(sel1)[0]])
        w = np.concatenate([wts[sel0, 0], wts[sel1, 1]])
        idxs.append(idx)
        ws.append(w)
    counts = [len(i) for i in idxs]

    # Assign experts to the G=2 token groups so slot i of both groups
    # has near-equal load: sort by count desc, alternate groups. Slot
    # capacity = pairwise max, so both groups share one chunk plan
    # (SPMD needs a single program) with minimal zero-padding.
    rank = sorted(range(E), key=lambda e: -counts[e])
    groups = [rank[0::2], rank[1::2]]  # [G][NS] expert ids, desc count
    slot_counts = [max(counts[groups[0][i]], counts[groups[1][i]])
                   for i in range(NS)]

    nc, CT = _get_nc(slot_counts)
    plan_offs = []
    off = 0
    for sc in slot_counts:
        plan_offs.append(off)
        off += sc

    xt16 = x.T.astype(np.float16)  # [D, T]
    xg, wins, wouts, bins = [], [], [], []
    for g in range(G):
        x8 = np.zeros((D, CT), dtype=np.float16)
        for i, e in enumerate(groups[g]):
            x8[:, plan_offs[i]:plan_offs[i] + counts[e]] = xt16[:, idxs[e]]
        xg.append(x8)
    for c in range(NCORES):
        g, s = c // NS, c % NS
        sl = slice(s * FS, (s + 1) * FS)
        es = groups[g]
        wins.append(np.ascontiguousarray(
            W_in[es][:, :, sl].transpose(1, 0, 2).astype(np.float16)))
        wouts.append(np.ascontiguousarray(
            W_out[es][:, sl, :].transpose(1, 0, 2).astype(np.float16)))
        bins.append(np.ascontiguousarray(
            b_in[es][:, sl].reshape(NS, NB, P).transpose(2, 0, 1)
            .reshape(P, NS * NB)))

    in_maps = [{"x8": xg[c // NS], "win8": wins[c], "wout8": wouts[c],
                "bin8": bins[c]} for c in range(NCORES)]

    if os.environ.get("BASS_TRACE"):
        _install_ntff_hook()
    LAST_RESULT = run_bass_kernel_spmd(nc, in_maps, list(range(NCORES)))

    # Host reduction: per group, sum the 4 F-slice partials, add b_out,
    # apply the top-2 combine weights, scatter-add into [T, D].
    y = np.zeros((T, D), dtype=np.float32)
    for g in range(G):
        Y = np.zeros((D, CT), dtype=np.float32)
        for s in range(NS):
            Y += LAST_RESULT.results[g * NS + s]["y8"].astype(np.float32)
        for i, e in enumerate(groups[g]):
            cnt = counts[e]
            cols = Y[:, plan_offs[i]:plan_offs[i] + cnt] + b_out[e][:, None]
            y[idxs[e]] += (cols * ws[e][None, :]).T
    return y.reshape(B, S, D)


# revision 17
# speedup vs baseline: 1.0249x; 1.0132x over previous
"""MoE MLP (top-2 routing, 8 experts) on 8 Trainium2 NeuronCores.

Strategy: 2 token groups x 4-way F-split (hybrid of data- and
tensor-parallel over the expert MLPs). One-expert-per-core is
load-imbalanced (busiest expert 2175 vs mean 2048 of the 16384
token-expert pairs; exec time is the max over cores). Pure 8-way
F-split balances perfectly but ships the full 16384-token stream and
a full-size y partial to/from every core — the ~900MB/call of
host<->device traffic overlaps neighboring cores' execution on shared
HBM and produced 10-50us random stragglers. The G=2 x S=4 hybrid
halves both: cores 0-3 process token group A against F-slices
0-3 (1024 wide) of group A's 4 experts; cores 4-7 likewise for
group B. Experts are assigned to groups by sorted load (alternating),
and slot capacities are the pairwise max, so both groups share ONE
chunk plan (SPMD requires a single program) with only ~1% padding:
8285 tokens/core x (1024x1024) x 2 matmuls ~= 442us fp16 PE roofline.

The router (0.05% of FLOPs) runs on the host, which doubles as the
dispatch: tokens are gathered per selected expert into a [D, 8285]
fp16 stream per group. Each core runs the fused MLP

    y_partial = W_out[sl,:]^T @ gelu(W_in[:,sl]^T @ x + b_in[sl])

in a transposed layout (tokens on the free axis) so weights stay
stationary on the PE array. Combine weights and b_out are applied on
the host during the partial-sum reduction (free), so the device does
no per-token scaling at all. Weight slot slices (4MB) stream on the
sync DMA ring two slots ahead; x chunks ride the scalar ring; y
partials (fp16) interleave on the sync ring.

fp8 was evaluated and rejected: e4m3 quantization measures 4.6e-2
rel-max error on this problem (gate is 2e-2); every partial-fp8
scheme also fails. fp16 measures ~4e-4.
"""

import contextlib
import ctypes
import os
import sys
import types
from contextlib import ExitStack

import numpy as np

import concourse.bass as bass
import concourse.mybir as mybir
import concourse.tile as tile
from concourse import bacc
from concourse.bass_utils import run_bass_kernel_spmd


def _install_ntff_hook():
    """Provide antenv.axon_hooks (absent in this image) so BASS_TRACE=1
    can capture NTFF profiles through the axon PJRT .so. No-op if the
    module already exists or the .so/symbols are unavailable."""
    try:
        from antenv.axon_hooks import get_axon_ntff_profile_hook  # noqa: F401
        return
    except ImportError:
        pass
    so_path = "/opt/axon/libaxon_pjrt.so"
    if not os.path.exists(so_path):
        return
    try:
        lib = ctypes.CDLL(so_path)
    except OSError:
        return
    if not hasattr(lib, "axon_start_nrt_profile"):
        return
    lib.axon_start_nrt_profile.argtypes = [
        ctypes.POINTER(ctypes.c_int64), ctypes.c_size_t]
    lib.axon_start_nrt_profile.restype = ctypes.c_int64
    lib.axon_stop_nrt_profile.argtypes = [ctypes.c_char_p]
    lib.axon_stop_nrt_profile.restype = ctypes.c_int64

    @contextlib.contextmanager
    def _hook(output_dir, device_ids):
        import jax
        jax.devices()  # force PJRT init so the .so's client exists
        if device_ids:
            ids = (ctypes.c_int64 * len(device_ids))(*device_ids)
            rc = lib.axon_start_nrt_profile(ids, len(device_ids))
        else:
            rc = lib.axon_start_nrt_profile(None, 0)
        if rc != 0:
            raise RuntimeError(f"axon_start_nrt_profile rc={rc}")
        try:
            yield
        finally:
            n = lib.axon_stop_nrt_profile(str(output_dir).encode())
            print(f"ntff profile: {n} file(s) -> {output_dir}", file=sys.stderr)

    import antenv
    mod = types.ModuleType("antenv.axon_hooks")
    mod.get_axon_ntff_profile_hook = lambda: _hook
    mod.set_axon_ntff_profile_hook = lambda h: None
    sys.modules["antenv.axon_hooks"] = mod
    antenv.axon_hooks = mod

B, S, D, F, E = 4, 2048, 1024, 4096, 8
T = B * S
TOP_K = 2
NCORES = 8
P = 128
G, NS = 2, 4              # 2 token groups x 4-way F-split (4 expert slots/core)
FS = F // NS              # 1024-wide F-slice per core
ND, NB = D // P, FS // P  # 8 d-tiles, 8 f-tiles per slice

# test.py pokes these for profiling info
LAST_RESULT = None

_cache = {}


def _chunk_list(C):
    """Token chunks (PSUM free-dim <= 512). Chunks below 256 run
    LDWEIGHTS-bound on the PE, so a short tail is split off the
    previous 512 chunk into two >=256 pieces instead."""
    chunks = [512] * (C // 512)
    rem = C % 512
    if rem:
        if rem < 256 and chunks:
            tot = 512 + rem
            a = tot // 2
            chunks[-1] = a
            chunks.append(tot - a)
        else:
            chunks.append(rem)
    return chunks


def _chunk_plan(counts):
    """[(expert, global_off, size, first_of_expert)], with a small ramp
    chunk up front (fast time-to-first-matmul) and a small tail chunk
    (short drain after the last matmul)."""
    lists = [_chunk_list(c) for c in counts]
    for lst in lists:
        if lst:
            if lst[0] >= 512:  # startup ramp: 128+384 instead of 512
                lst[0] = 384
                lst.insert(0, 128)
            break
    for lst in reversed(lists):
        if lst:
            if lst[-1] > 192:  # tail: split off a final 128
                lst[-1] -= 128
                lst.append(128)
            break
    plan = []
    off = 0
    for e, lst in enumerate(lists):
        for i, ck in enumerate(lst):
            plan.append((e, off, ck, i == 0))
            off += ck
    return plan, off


def _build_bass(counts):
    dt = mybir.dt
    io_dt = dt.float16
    plan, CT = _chunk_plan(counts)
    nc = bacc.Bacc("TRN2", target_bir_lowering=False, debug=False)

    x8 = nc.dram_tensor("x8", [D, CT], io_dt, kind="ExternalInput")
    win8 = nc.dram_tensor("win8", [D, NS, FS], io_dt, kind="ExternalInput")
    wout8 = nc.dram_tensor("wout8", [FS, NS, D], io_dt, kind="ExternalInput")
    # host pre-arranges b_in slice as [p, slot*fc] so the DMA is contiguous
    bin8 = nc.dram_tensor("bin8", [P, NS * NB], dt.float32, kind="ExternalInput")
    y8 = nc.dram_tensor("y8", [D, CT], io_dt, kind="ExternalOutput")

    x_r = x8.ap().rearrange("(dn p) c -> p dn c", p=P)
    win_r = win8.ap().rearrange("(dn p) e f -> p dn e f", p=P)
    wout_r = wout8.ap().rearrange("(fb p) e d -> p fb e d", p=P)
    y_r = y8.ap().rearrange("(dn p) c -> p dn c", p=P)

    with tile.TileContext(nc) as tc, ExitStack() as ctx:
        consts = ctx.enter_context(tc.tile_pool(name="consts", bufs=1))
        xpool = ctx.enter_context(tc.tile_pool(name="x", bufs=3))
        winpool = ctx.enter_context(tc.tile_pool(name="win", bufs=3))
        woutpool = ctx.enter_context(tc.tile_pool(name="wout", bufs=3))
        hpool = ctx.enter_context(tc.tile_pool(name="h", bufs=2))
        ypool = ctx.enter_context(tc.tile_pool(name="y", bufs=16))
        psum_h = ctx.enter_context(tc.tile_pool(name="ph", bufs=3, space="PSUM"))
        psum_y = ctx.enter_context(tc.tile_pool(name="py", bufs=3, space="PSUM"))

        def x_dma(e_off_ck):
            _, off, ck, _ = e_off_ck
            x_t = xpool.tile([P, ND, ck], io_dt, tag="x")
            nc.scalar.dma_start(x_t[:], x_r[:, :, off:off + ck])
            return x_t

        def w_dma(e):
            win_t = winpool.tile([P, ND, FS], io_dt, tag="win")
            nc.sync.dma_start(win_t[:], win_r[:, :, e, :])
            wout_t = woutpool.tile([P, NB, D], io_dt, tag="wout")
            nc.sync.dma_start(wout_t[:], wout_r[:, :, e, :])
            return win_t, wout_t

        # Critical path: x chunk 0 (scalar ring) and slot 0's W_in
        # (sync ring, f-halves: the first 4 psum stripes of chunk 0
        # only wait on half the load; the rest lands while they run).
        x0_t = xpool.tile([P, ND, plan[0][2]], io_dt, tag="x", name="x0")
        nc.scalar.dma_start(x0_t[:], x_r[:, :, 0:plan[0][2]])
        x_pref = [x0_t]
        win0_t = winpool.tile([P, ND, FS], io_dt, tag="win")
        nc.sync.dma_start(win0_t[:, :, :FS // 2], win_r[:, :, 0, :FS // 2])
        nc.sync.dma_start(win0_t[:, :, FS // 2:], win_r[:, :, 0, FS // 2:])
        wout0_t = woutpool.tile([P, NB, D], io_dt, tag="wout")
        nc.sync.dma_start(wout0_t[:], wout_r[:, :, 0, :])
        w_tiles = {0: (win0_t, wout0_t)}

        # b_in is tiny and needed by the first gelu — SWDGE queue.
        bin_t = consts.tile([P, NS * NB], dt.float32)
        nc.gpsimd.dma_start(bin_t[:], bin8.ap())

        # Prefetch: x one chunk ahead, weights one slot ahead.
        if len(plan) > 1:
            x_pref.append(x_dma(plan[1]))
        if NS > 1 and counts[1]:
            w_tiles[1] = w_dma(1)

        def m1(ci):
            # phase A: h = gelu(W_in^T @ x + b_in), laid out [f, tok]
            e, off, ck, first = plan[ci]
            x_t = x_pref.pop(0)
            if ci + 2 < len(plan):
                x_pref.append(x_dma(plan[ci + 2]))
            if e not in w_tiles:  # fallback for degenerate chunk plans
                w_tiles[e] = w_dma(e)
            win_t = w_tiles[e][0]
            h_t = hpool.tile([P, NB, ck], io_dt, tag="h", name="h_t")
            for fc in range(NB):
                ph = psum_h.tile([P, ck], dt.float32, tag="ph", name="ph")
                for dn in range(ND):
                    nc.tensor.matmul(
                        ph[:],
                        win_t[:, dn, fc * P:(fc + 1) * P],
                        x_t[:, dn, :],
                        start=(dn == 0),
                        stop=(dn == ND - 1),
                    )
                nc.scalar.activation(
                    h_t[:, fc, :], ph[:],
                    mybir.ActivationFunctionType.Gelu,
                    bias=bin_t[:, e * NB + fc:e * NB + fc + 1],
                )
            return h_t

        def m2(ci, h_t):
            # phase B: y_partial = W_out^T @ h, laid out [d, tok]
            e, off, ck, first = plan[ci]
            if not first and plan[ci - 1][3] and e + 2 < NS and counts[e + 2] \
                    and (e + 2) not in w_tiles:
                # second chunk of slot e: prefetch slot e+2's weights
                # (queued behind the first chunk's y DMAs on the sync ring)
                w_tiles[e + 2] = w_dma(e + 2)
            wout_t = w_tiles[e][1]
            last = ci >= len(plan) - 2  # drain tail on both rings
            for dn in range(ND):
                py = psum_y.tile([P, ck], dt.float32, tag="py", name="py")
                for fb in range(NB):
                    nc.tensor.matmul(
                        py[:],
                        wout_t[:, fb, dn * P:(dn + 1) * P],
                        h_t[:, fb, :],
                        start=(fb == 0),
                        stop=(fb == NB - 1),
                    )
                y_t = ypool.tile([P, ck], io_dt, tag="y", name="y_t")
                nc.vector.tensor_copy(y_t[:], py[:])
                eng = nc.scalar if (last and dn % 2) else nc.sync
                eng.dma_start(y_r[:, dn, off:off + ck], y_t[:])

        # Software-pipeline m1 one chunk ahead of m2: m2(k) overlaps
        # m1(k+1)'s matmuls and covers each expert's W_out DMA latency
        # (chunk 0's m2 would otherwise stall on wout0 at startup).
        h_prev = m1(0)
        for ci in range(len(plan)):
            h_cur = h_prev
            if ci + 1 < len(plan):
                h_prev = m1(ci + 1)
            m2(ci, h_cur)

    nc.compile()
    return nc, CT


def _get_nc(counts):
    key = tuple(counts)
    if key not in _cache:
        _cache[key] = _build_bass(counts)
    return _cache[key]


def _route(x, W_router):
    """Host-side router: top-2 selection + renormalized weights (fp64).

    Matches jax.lax.top_k on softmax(logits): softmax is monotone so
    top-2 of logits is identical, with ties broken toward lower index
    (argsort stable on -logits).
    """
    lg = x.astype(np.float64) @ W_router.T.astype(np.float64)
    top2 = np.argsort(-lg, axis=1, kind="stable")[:, :TOP_K]
    l1 = np.take_along_axis(lg, top2[:, 0:1], 1)
    l2 = np.take_along_axis(lg, top2[:, 1:2], 1)
    e2 = np.exp(l2 - l1)
    w1 = (1.0 / (1.0 + e2)).astype(np.float32)
    w2 = (e2 / (1.0 + e2)).astype(np.float32)
    return top2, np.concatenate([w1, w2], axis=1)


def kernel(residual, W_router, W_in, b_in, W_out, b_out):
    global LAST_RESULT

    x = np.ascontiguousarray(np.asarray(residual, dtype=np.float32).reshape(T, D))
    W_in = np.asarray(W_in, dtype=np.float32)
    W_out = np.asarray(W_out, dtype=np.float32)
    b_in = np.asarray(b_in, dtype=np.float32)
    b_out = np.asarray(b_out, dtype=np.float32)

    top2, wts = _route(x, np.asarray(W_router, dtype=np.float32))

    idxs, ws = [], []
    for e in range(E):
        sel0 = top2[:, 0] == e
        sel1 = top2[:, 1] == e
        idx = np.concatenate([np.where(sel0)[0], np.where(sel1)[0]])
        w = np.concatenate([wts[sel0, 0], wts[sel1, 1]])
        idxs.append(idx)
        ws.append(w)
    counts = [len(i) for i in idxs]

    # Assign experts to the G=2 token groups so slot i of both groups
    # has near-equal load: sort by count desc, alternate groups. Slot
    # capacity = pairwise max, so both groups share one chunk plan
    # (SPMD needs a single program) with minimal zero-padding.
    rank = sorted(range(E), key=lambda e: -counts[e])
    groups = [rank[0::2], rank[1::2]]  # [G][NS] expert ids, desc count
    slot_counts = [max(counts[groups[0][i]], counts[groups[1][i]])
                   for i in range(NS)]

    nc, CT = _get_nc(slot_counts)
    plan_offs = []
    off = 0
    for sc in slot_counts:
        plan_offs.append(off)
        off += sc

    xt16 = x.T.astype(np.float16)  # [D, T]
    xg, wins, wouts, bins = [], [], [], []
    for g in range(G):
        x8 = np.zeros((D, CT), dtype=np.float16)
        for i, e in enumerate(groups[g]):
            x8[:, plan_offs[i]:plan_offs[i] + counts[e]] = xt16[:, idxs[e]]
        xg.append(x8)
    for c in range(NCORES):
        g, s = c // NS, c % NS
        sl = slice(s * FS, (s + 1) * FS)
        es = groups[g]
        wins.append(np.ascontiguousarray(
            W_in[es][:, :, sl].transpose(1, 0, 2).astype(np.float16)))
        wouts.append(np.ascontiguousarray(
            W_out[es][:, sl, :].transpose(1, 0, 2).astype(np.float16)))
        bins.append(np.ascontiguousarray(
            b_in[es][:, sl].reshape(NS, NB, P).transpose(2, 0, 1)
            .reshape(P, NS * NB)))

    in_maps = [{"x8": xg[c // NS], "win8": wins[c], "wout8": wouts[c],
                "bin8": bins[c]} for c in range(NCORES)]

    if os.environ.get("BASS_TRACE"):
        _install_ntff_hook()
    LAST_RESULT = run_bass_kernel_spmd(nc, in_maps, list(range(NCORES)))

    # Host reduction: per group, sum the 4 F-slice partials, add b_out,
    # apply the top-2 combine weights, scatter-add into [T, D].
    y = np.zeros((T, D), dtype=np.float32)
    for g in range(G):
        Y = np.zeros((D, CT), dtype=np.float32)
        for s in range(NS):
            Y += LAST_RESULT.results[g * NS + s]["y8"].astype(np.float32)
        for i, e in enumerate(groups[g]):
            cnt = counts[e]
            cols = Y[:, plan_offs[i]:plan_offs[i] + cnt] + b_out[e][:, None]
            y[idxs[e]] += (cols * ws[e][None, :]).T
    return y.reshape(B, S, D)
